# revision 99
# baseline (speedup 1.0000x reference)
"""Causal self-attention with RoPE on 8 Trainium2 NeuronCores.

Sharding: tensor-parallel over heads (16 heads -> 2 per core) for the
QKV projections, RoPE and attention.  The attention output is
re-sharded token-wise with one small AllToAll per 512-token chunk
(bf16 payload), so each core ends up with the full 2048-dim y vector
for 64 tokens of every chunk; the output projection then runs
token-parallel with no all-reduce.

Single pass over the sequence: both heads' q/k/v are produced from one
streaming of x (the baseline used one pass per head and loaded x
twice).

Key engine-placement choices (cost-model driven):
  - softmax denominator l = sum_k p is NOT computed with PE matmuls
    (those cost as much as the score matmuls); instead p-blocks are
    summed with a shallow tree of tensor_tensor adds spread across
    DVE/Pool and one gpsimd.partition_all_reduce, whose output is
    already broadcast across partitions (also kills the r-broadcast
    matmul of the baseline).
  - y, v and W_o are bf16 (halves the collective payload and the Wo
    weight traffic; bf16 matmuls run at full PE rate).
  - W_o is loaded ONCE into 16 resident SBUF tiles (8MB of HBM
    traffic instead of 16MB) shared by all three output-projection
    passes; the loads are anchored behind chunk 5's x load so they
    never steal DMA bandwidth from the startup-critical stream.
  - startup: weights lead on the SP queue (wq g0 gates the first
    matmul), x chunk 0 streams on the Act queue, wv/rope ride the
    Pool SWDGE path; chunk 0 runs three projection chains
    kb-interleaved so PE tracks the arriving DMA stream, with the
    fourth chain and two v-chains borrowing idle attention PSUM
    banks to dodge the ps_proj rotation wait.

Shapes (hardcoded): x [2, 2048, 2048], W_* [2048, 2048], 16 heads,
d_k = 128, fp32 in/out.
"""

import sys

for _p in ("/opt/trn_rl_repo", "/opt/pypackages"):
    if _p not in sys.path:
        sys.path.insert(0, _p)

import numpy as np

import concourse.bass as bass
import concourse.bacc as bacc
import concourse.mybir as mybir
import concourse.tile as tile
from concourse import bass_utils
from concourse import bass_isa
from concourse.alu_op_type import AluOpType
from concourse.tile import add_dep_helper

# ---------------------------------------------------------------- config
N_CORES = 8
B, S, D = 2, 2048, 2048
H = 16
DK = D // H              # 128
HPC = H // N_CORES       # 2 heads per core
TOK = B * S              # 4096
SUB = 512                # token sub-chunk for projections (= one chunk)
QCH = 512                # attention query chunk
JB = 128                 # attention key block
NSUB = TOK // SUB        # 16
NCH = TOK // QCH         # 8 query chunks
KB = D // 128            # 16 contraction blocks
TPC = QCH // N_CORES     # 64 tokens per (chunk, core) after AllToAll
ROPE_BASE = 10000.0
MASK_NEG = -30000.0

F32 = mybir.dt.float32
F32R = mybir.dt.float32r
BF16 = mybir.dt.bfloat16

_CACHE = {}


def _build_nc():
    dt = F32R
    nc = bacc.Bacc("TRN2", target_bir_lowering=False, debug=False,
                   num_devices=N_CORES)

    xT = nc.dram_tensor("xT", [D, TOK], BF16, kind="ExternalInput")
    wqT = nc.dram_tensor("wqT", [D, HPC * DK], BF16, kind="ExternalInput")
    wkT = nc.dram_tensor("wkT", [D, HPC * DK], BF16, kind="ExternalInput")
    wvT = nc.dram_tensor("wvT", [D, HPC * DK], BF16, kind="ExternalInput")
    # W_o.T in bf16, tiled (eb, p, dl, e'): row = eb*2048 + p*16 + dl
    woT = nc.dram_tensor("woT", [KB * D, DK], BF16, kind="ExternalInput")
    ropeC = nc.dram_tensor("ropeC", [DK, S], BF16, kind="ExternalInput")
    ropeS = nc.dram_tensor("ropeS", [DK, S], BF16, kind="ExternalInput")
    maskd = nc.dram_tensor("maskd", [JB, QCH + 384], BF16, kind="ExternalInput")
    # out columns ordered (jc 0..7, t 0..63): global token 512*jc + 64*c + t
    outT = nc.dram_tensor("outT", [D, QCH], BF16, kind="ExternalOutput")

    swap_mask = [i ^ 1 for i in range(32)]

    import contextlib
    with tile.TileContext(nc) as tc:
        with contextlib.ExitStack() as st:
            dram = st.enter_context(
                tc.tile_pool(name="dram", bufs=1, space="DRAM"))
            a2a_in = [dram.tile([N_CORES * HPC * DK, TPC], BF16,
                                name=f"a2ain{j}") for j in range(NCH)]
            a2a_out = [dram.tile([N_CORES * HPC * DK, TPC], BF16,
                                 name=f"a2aout{j}") for j in range(NCH)]

            const = st.enter_context(tc.tile_pool(name="const", bufs=1))
            xpool = st.enter_context(
                tc.tile_pool(name="xpool", bufs=2, side="right"))
            qpool = st.enter_context(
                tc.tile_pool(name="qpool", bufs=2, side="right"))
            kvpool = st.enter_context(
                tc.tile_pool(name="kvpool", bufs=8, side="right"))
            vpool = st.enter_context(
                tc.tile_pool(name="vpool", bufs=15, side="right"))
            ppool = st.enter_context(tc.tile_pool(name="ppool", bufs=8))
            lpool = st.enter_context(tc.tile_pool(name="lpool", bufs=2))
            work = st.enter_context(tc.tile_pool(name="work", bufs=2))
            ypool = st.enter_context(tc.tile_pool(name="ypool", bufs=3))
            ytpool = st.enter_context(tc.tile_pool(name="ytpool", bufs=1))
            stpool = st.enter_context(tc.tile_pool(name="stpool", bufs=1))
            wopool = st.enter_context(
                tc.tile_pool(name="wopool", bufs=16, side="right"))
            ps_proj = st.enter_context(
                tc.tile_pool(name="ps_proj", bufs=3, space="PSUM"))
            ps_st = st.enter_context(
                tc.tile_pool(name="ps_st", bufs=3, space="PSUM"))
            ps_out = st.enter_context(
                tc.tile_pool(name="ps_out", bufs=2, space="PSUM"))

            # chunk-0 xT goes first on its queues so the first projection
            # isn't stuck behind weight DMAs
            def xt_dma(xt, sc):
                KH = KB // 2
                di = None
                for xh in range(2):
                    di = nc.sync.dma_start(
                        xt[:, xh * KH * SUB:(xh + 1) * KH * SUB]
                          .rearrange("p (kb t) -> p kb t", kb=KH),
                        xT.ap()[xh * KH * 128:(xh + 1) * KH * 128,
                                sc * SUB:(sc + 1) * SUB]
                          .rearrange("(kb p) t -> p kb t", p=128))
                return di

            xt_anchor = {}
            store_anchor = {}
            a2a_inst = {}

            # ---- persistent constants in SBUF (tiles; DMAs below)
            wq_sb = const.tile([128, KB * HPC * DK], BF16)
            wk_sb = const.tile([128, KB * HPC * DK], BF16)
            wv_sb = const.tile([128, KB * HPC * DK], BF16)
            ropeC_sb = const.tile([DK, S], BF16)
            ropeS_sb = const.tile([DK, S], BF16)
            maskd_sb = const.tile([JB, QCH + 384], BF16)

            def w_dma(eng, sb_t, dr, kb0, nkb):
                m0 = kb0 * HPC * DK
                return eng.dma_start(
                    sb_t[:, m0:m0 + nkb * HPC * DK]
                        .rearrange("p (kb m) -> p kb m", kb=nkb),
                    dr.ap()[kb0 * 128:(kb0 + nkb) * 128, :]
                      .rearrange("(kb p) m -> p kb m", p=128))

            # weights lead on the SP queue (565ns DMA-SEQ vs Act's 667 --
            # wq g0 gates the very first matmul); x chunk 0 streams on Act.
            GW = 4
            for g in range(KB // GW):
                w_dma(nc.sync, wq_sb, wqT, g * GW, GW)
                w_dma(nc.sync, wk_sb, wkT, g * GW, GW)

            xt0 = xpool.tile([128, KB * SUB], BF16, tag="xt", name="xt")
            KH0 = KB // 8
            for xh in range(8):
                di = nc.scalar.dma_start(
                    xt0[:, xh * KH0 * SUB:(xh + 1) * KH0 * SUB]
                       .rearrange("p (kb t) -> p kb t", kb=KH0),
                    xT.ap()[xh * KH0 * 128:(xh + 1) * KH0 * 128, 0:SUB]
                      .rearrange("(kb p) t -> p kb t", p=128))
            xt_anchor[0] = di
            w_dma(nc.gpsimd, wv_sb, wvT, 0, 4)
            nc.gpsimd.dma_start(ropeC_sb[:, :SUB], ropeC[:, :SUB])
            nc.gpsimd.dma_start(ropeS_sb[:, :SUB], ropeS[:, :SUB])
            for g in range(1, KB // GW):
                w_dma(nc.gpsimd, wv_sb, wvT, g * GW, GW)
            nc.scalar.dma_start(ropeC_sb[:, SUB:], ropeC[:, SUB:])
            nc.scalar.dma_start(ropeS_sb[:, SUB:], ropeS[:, SUB:])
            nc.scalar.dma_start(maskd_sb[:], maskd[:])

            def rope_combine(ps_in, out_ap, s0, n, t1_eng=None):
                """out = ps_in * C + shuffle(ps_in) * S  (RoPE).

                t1_eng pins the second PSUM-reading op so the PSUM buffer
                frees as early as possible (used on chunk 0's critical
                startup path)."""
                qsh = work.tile([128, SUB], F32, tag="qsh", name="qsh")
                t1 = work.tile([128, SUB], BF16, tag="t1", name="t1")
                nc.vector.stream_shuffle(qsh[:, :n], ps_in, swap_mask)
                (t1_eng or nc.any).tensor_tensor(
                    t1[:, :n], ps_in, ropeC_sb[:, s0:s0 + n], AluOpType.mult)
                nc.vector.tensor_tensor(
                    qsh[:, :n], qsh[:, :n], ropeS_sb[:, s0:s0 + n],
                    AluOpType.mult)
                nc.any.tensor_tensor(out_ap, t1[:, :n], qsh[:, :n],
                                     AluOpType.add)

            chain_engs = [nc.vector, nc.gpsimd]

            # Wo token-passes -------------------------------------------------
            # pass 0: chunks 0-3 (cols 0:256), interleaved into the second
            # half of the main loop; pass 1: chunks 4-6 (cols 256:448);
            # pass 2: chunk 7 (cols 448:512), reusing resident wo tiles.
            wo_engs = [nc.scalar, nc.scalar]

            def load_woeb(eb, ei, anchor=None):
                wo_eb = wopool.tile([128, KB * DK], BF16, tag="wo",
                                    name="wo_eb")
                di = wo_engs[ei % 2].dma_start(
                    wo_eb[:],
                    woT.ap()[eb * D:(eb + 1) * D, :]
                       .rearrange("(p dl) e -> p (dl e)", p=128))
                if anchor is not None:
                    add_dep_helper(di.ins, anchor.ins, sync=True,
                                   reason="wo load after main loop")
                return wo_eb

            def load_yt(yt, chunks, ci0=0, anchors=None, eng=None):
                for ci, jc in enumerate(chunks):
                    di = (eng or nc.scalar).dma_start(
                        yt.rearrange("p (db c) -> p db c", db=KB)
                          [:, :, (ci0 + ci) * TPC:(ci0 + ci + 1) * TPC],
                        a2a_out[jc].rearrange("(db p) t -> p db t",
                                                   p=128))
                    if anchors is not None:
                        add_dep_helper(di.ins, anchors[ci].ins, sync=True,
                                       reason="yt load after its A2A landed")

            def wo_eb_mm(eb, yt, ncols, stage, wo_eb=None, ei=0,
                         copy_eng=None):
                if wo_eb is None:
                    wo_eb = load_woeb(eb, ei)
                ps_w = ps_proj.tile([128, SUB], F32, tag="proj", name="ps_w")
                for dl in range(KB):
                    nc.tensor.matmul(
                        ps_w[:, :ncols],
                        wo_eb[:, dl * DK:(dl + 1) * DK],
                        yt[:, dl * ncols:(dl + 1) * ncols],
                        start=(dl == 0), stop=(dl == KB - 1))
                (copy_eng or nc.any).tensor_copy(
                    stage[:, eb * ncols:(eb + 1) * ncols], ps_w[:, :ncols])

            def store_stage(stage, ncols, col0):
                nc.sync.dma_start(
                    outT.ap().rearrange("(eb p) c -> p eb c", p=128)
                        [:, :, col0:col0 + ncols],
                    stage.rearrange("p (eb c) -> p eb c", eb=KB))

            # ---------------- main pass over the sequence -------------------
            qT_tiles = {}
            kT_tiles = {}
            v_tiles = {}
            yt_p0 = [None]

            for sc in range(NSUB):
                b = sc // (NSUB // B)
                iq = sc % (NSUB // B)
                jc_glob = sc
                s0 = iq * SUB                   # position within batch

                if sc == 0:
                    xt = xt0
                else:
                    xt = xpool.tile([128, KB * SUB], BF16, tag="xt", name="xt")
                    xt_anchor[sc] = xt_dma(xt, sc)

                # ---- q/k projections + rope, both heads
                for h in range(HPC):
                    qT_tiles[h] = qpool.tile([128, QCH], BF16,
                                             tag=f"qT{h}", name="qT")
                    kT_tiles[(b, h, iq)] = kvpool.tile(
                        [128, QCH], BF16, tag="kT", name="kT")
                if sc == 0:
                    # chunk 0: x/weights are still streaming in.  Run three
                    # chains kb-interleaved (one matmul per arriving
                    # kb-group each) so PE tracks the DMA stream instead of
                    # idling through the first full chain.
                    specs = [(wq_sb, 0, qT_tiles[0]),
                             (wq_sb, 1, qT_tiles[1]),
                             (wk_sb, 0, kT_tiles[(b, 0, iq)])]
                    ps3 = [ps_proj.tile([128, SUB], F32, tag="proj",
                                        name="proj") for _ in specs]
                    psv_il = ps_st.tile([JB, QCH], F32, tag="st", name="st")
                    for kb in range(KB):
                        for ci, (w_sb, h, _) in enumerate(specs):
                            nc.tensor.matmul(
                                ps3[ci][:],
                                w_sb[:, kb * HPC * DK + h * DK:
                                     kb * HPC * DK + (h + 1) * DK],
                                xt[:, kb * SUB:(kb + 1) * SUB],
                                start=(kb == 0), stop=(kb == KB - 1))
                            if ci == 0:
                                nc.tensor.matmul(
                                    psv_il[:, :HPC * DK],
                                    xt[:, kb * SUB:kb * SUB + 128],
                                    wv_sb[:, kb * HPC * DK:
                                          (kb + 1) * HPC * DK],
                                    start=(kb == 0), stop=(kb == KB - 1))
                    psk1 = ps_st.tile([JB, QCH], F32, tag="st", name="st")
                    for kb in range(KB):
                        nc.tensor.matmul(
                            psk1[:],
                            wk_sb[:, kb * HPC * DK + DK:
                                 kb * HPC * DK + 2 * DK],
                            xt[:, kb * SUB:(kb + 1) * SUB],
                            start=(kb == 0), stop=(kb == KB - 1))
                    for ci, (_, _, dst) in enumerate(specs):
                        rope_combine(ps3[ci][:], dst[:], s0, SUB)
                    rope_combine(psk1[:], kT_tiles[(b, 1, iq)][:], s0, SUB)
                    vt = vpool.tile([128, HPC * DK], BF16, tag="v", name="vt")
                    nc.any.tensor_copy(vt[:], psv_il[:, :HPC * DK])
                    v_tiles[(b, 0)] = vt
                    rem = []
                else:
                    rem = [(wq_sb, 0, qT_tiles[0]),
                           (wk_sb, 0, kT_tiles[(b, 0, iq)]),
                           (wq_sb, 1, qT_tiles[1]),
                           (wk_sb, 1, kT_tiles[(b, 1, iq)])]
                for ri, (w_sb, h, dst) in enumerate(rem):
                    # the last chain borrows an (idle until attention)
                    # ps_st buffer, cutting the ps_proj rotation wait
                    if ri == 3:
                        psq = ps_st.tile([JB, QCH], F32, tag="st", name="st")
                    else:
                        psq = ps_proj.tile([128, SUB], F32, tag="proj",
                                           name="proj")
                    for kb in range(KB):
                        nc.tensor.matmul(
                            psq[:],
                            w_sb[:, kb * HPC * DK + h * DK:
                                 kb * HPC * DK + (h + 1) * DK],
                            xt[:, kb * SUB:(kb + 1) * SUB],
                            start=(kb == 0), stop=(kb == KB - 1))
                    rope_combine(psq[:], dst[:], s0, SUB)

                # ---- v projection, both heads, bf16 storage
                for tb in range(1 if sc == 0 else 0, SUB // 128):
                    jb_b = iq * (SUB // 128) + tb
                    psv = (ps_st.tile([JB, QCH], F32, tag="st", name="st")
                           if tb < 2 else
                           ps_proj.tile([128, HPC * DK], F32, tag="proj",
                                        name="psv"))
                    for kb in range(KB):
                        nc.tensor.matmul(
                            psv[:, :HPC * DK],
                            xt[:, kb * SUB + tb * 128:
                               kb * SUB + (tb + 1) * 128],
                            wv_sb[:, kb * HPC * DK:(kb + 1) * HPC * DK],
                            start=(kb == 0), stop=(kb == KB - 1))
                    vt = vpool.tile([128, HPC * DK], BF16, tag="v", name="vt")
                    nc.any.tensor_copy(vt[:], psv[:, :HPC * DK])
                    v_tiles[(b, jb_b)] = vt

                # ---- attention for the completed query chunk
                if True:
                    n_j = 4 * iq + 4
                    for h in range(HPC):
                        qT = qT_tiles[h]
                        ps_o = ps_out.tile([128, QCH], F32, tag="att_out", name="att_out")
                        p_tiles = {}

                        def emit_block(j):
                            jck, jr = j // 4, j % 4
                            m = j - 4 * iq
                            # diagonal block m: columns [0, 128m) are fully
                            # masked -> skip them and zero-fill p instead
                            q0 = 128 * m if m > 0 else 0
                            ps_s = ps_st.tile([JB, QCH], F32, tag="st", name="st")
                            nc.tensor.matmul(
                                ps_s[:, q0:],
                                kT_tiles[(b, h, jck)][:, jr * 128:
                                                      (jr + 1) * 128],
                                qT[:, q0:], start=True, stop=True)
                            if m >= 0:           # diagonal block: mask
                                # full width: the un-written [0,q0) region
                                # holds bounded stale scores; -30000 sends
                                # them to exp()==0, zero-filling p for free
                                # one [128, 896] extended mask serves all
                                # four diagonal blocks via shifted slices:
                                # mask_ext[k, 384-128m+q] == (k <= q-128m)
                                nc.vector.tensor_tensor(
                                    ps_s[:], ps_s[:],
                                    maskd_sb[:, 384 - 128 * m:
                                             384 - 128 * m + QCH],
                                    AluOpType.add)
                            p_t = ppool.tile([JB, QCH], BF16, tag="p", name="p")
                            nc.scalar.activation(
                                p_t[:], ps_s[:],
                                mybir.ActivationFunctionType.Exp)
                            p_tiles[j] = p_t

                        # scores/exp run two blocks ahead of the AV matmuls
                        # so PE never waits on the Act engine
                        emit_block(0)
                        if n_j > 1:
                            emit_block(1)
                        # l = sum_k p: two sequential accumulator chains
                        # (even/odd j) spread across DVE/Pool/Act
                        acc = [None, None]
                        n_add = 0
                        cengs = (chain_engs if jc_glob < NCH - 1
                                 else [nc.vector])
                        for j in range(n_j):
                            if j + 2 < n_j:
                                emit_block(j + 2)
                            p_t = p_tiles[j]
                            # diagonal blocks: columns [0,128m) of p are
                            # exact zeros -- skip them (j==0 is always
                            # full width, so the PSUM group opens whole)
                            m = j - 4 * iq
                            q0 = 128 * m if m > 0 else 0
                            nc.tensor.matmul(
                                ps_o[:, q0:], v_tiles[(b, j)][:, h * DK:
                                                              (h + 1) * DK],
                                p_t[:, q0:],
                                start=(j == 0), stop=(j == n_j - 1))
                            c = j % 2
                            if j >= 2:
                                eng = cengs[c % len(cengs)]
                                n_add += 1
                                if acc[c] is None:
                                    a_t = lpool.tile([128, QCH], BF16,
                                                     tag="acc", name="acc")
                                    eng.tensor_tensor(
                                        a_t[:], p_tiles[c][:], p_t[:],
                                        AluOpType.add)
                                    acc[c] = a_t
                                else:
                                    eng.tensor_tensor(
                                        acc[c][:], acc[c][:], p_t[:],
                                        AluOpType.add)
                        if acc[0] is None:       # n_j == 4 has j = 0..3
                            acc = [p_tiles[0], p_tiles[1]]
                        p_acc = lpool.tile([128, QCH], BF16, tag="acc2", name="acc2")
                        cengs[-1].tensor_tensor(
                            p_acc[:], acc[0][:], acc[1][:], AluOpType.add)
                        # gpsimd all-reduce output is broadcast across
                        # partitions -> no r-broadcast matmul needed
                        l_bc = lpool.tile([128, QCH], F32, tag="lbc", name="lbc")
                        nc.gpsimd.partition_all_reduce(
                            l_bc[:], p_acc[:], channels=128,
                            reduce_op=bass_isa.ReduceOp.add)
                        nc.vector.reciprocal_approx_fast(l_bc[:], l_bc[:])
                        y_sb = ypool.tile([128, QCH], BF16, tag="y", name="y")
                        nc.any.tensor_tensor(y_sb[:], ps_o[:], l_bc[:],
                                             AluOpType.mult)
                        # scatter into the AllToAll source layout:
                        # dst row m*256 + h*128 + p, col t <- src[p, m*64+t]
                        di = nc.gpsimd.dma_start(
                            a2a_in[jc_glob]
                            .rearrange("(m hp) t -> hp m t", m=N_CORES)
                            [h * 128:(h + 1) * 128],
                            y_sb.rearrange("p (m t) -> p m t", m=N_CORES))
                        if h == HPC - 1:
                            store_anchor[jc_glob] = di
                    a2a_inst[jc_glob] = nc.gpsimd.collective_compute(
                        "AllToAll", AluOpType.bypass,
                        replica_groups=[list(range(N_CORES))],
                        ins=[a2a_in[jc_glob].opt()],
                        outs=[a2a_out[jc_glob].opt()])


            # ---- Wo pass A: chunks 0-3 (cols 0:256).  Runs entirely in
            # the shadow of the last AllToAll: those chunks' y landed long
            # ago.  Every staging DMA is anchored so the dataflow
            # scheduler cannot hoist its wait into the main loop.
            yt_p0[0] = ytpool.tile([128, KB * 4 * TPC], BF16,
                                   tag="yt0", name="yt0")
            load_yt(yt_p0[0], range(4),
                    anchors=[store_anchor[c + 1] for c in range(4)])
            yt_p1 = ytpool.tile([128, KB * 3 * TPC], BF16, tag="yt1",
                                name="yt1")
            load_yt(yt_p1, range(4, 7),
                    anchors=[store_anchor[5], store_anchor[6],
                             store_anchor[7]])
            stage_a = stpool.tile([128, KB * 4 * TPC], BF16, tag="sta",
                                  name="sta")
            stage_b1 = stpool.tile([128, KB * 3 * TPC], BF16, tag="stb1",
                                   name="stb1")
            stage_b2 = stpool.tile([128, KB * TPC], BF16, tag="stb2",
                                   name="stb2")
            # The 16 wo tiles are loaded ONCE (8MB instead of 16MB of HBM
            # traffic) and stay resident through passes A, B1 and B2.
            wo_tiles = {eb: load_woeb(eb, eb, anchor=xt_anchor[5])
                        for eb in range(2)}
            for eb in range(KB):
                if eb + 2 < KB:       # stay 2 loads ahead of the matmuls
                    wo_tiles[eb + 2] = load_woeb(eb + 2, eb,
                                                 anchor=xt_anchor[5])
                wo_eb_mm(eb, yt_p0[0], 4 * TPC, stage_a,
                         wo_eb=wo_tiles[eb], ei=eb)

            store_stage(stage_a, 4 * TPC, 0)

            # ---- Wo pass B1: chunks 4-6 (cols 256:448) -- none of this
            # waits on the final AllToAll, so it fills the A2A-7 shadow
            # right after pass A.  Weights already resident.
            for eb in range(KB):
                wo_eb_mm(eb, yt_p1, 3 * TPC, stage_b1,
                         wo_eb=wo_tiles[eb], ei=eb)

            for eh in range(2):
                nc.sync.dma_start(
                    outT.ap().rearrange("(eb p) c -> p eb c", p=128)
                        [:, eh * 8:(eh + 1) * 8, 4 * TPC:7 * TPC],
                    stage_b1.rearrange("p (eb c) -> p eb c", eb=KB)
                            [:, eh * 8:(eh + 1) * 8])

            # ---- Wo pass B2: chunk 7 (cols 448:512), gated on the final
            # AllToAll.  Weights already resident.
            yt_p2 = ytpool.tile([128, KB * TPC], BF16, tag="yt2",
                                name="yt2")
            load_yt(yt_p2, range(7, 8),
                    anchors=[store_anchor[7]], eng=nc.gpsimd)
            for eb in range(KB):
                wo_eb_mm(eb, yt_p2, TPC, stage_b2, wo_eb=wo_tiles[eb],
                         copy_eng=nc.vector)
            # rows 3..15 (resident-weight blocks) finish first: store them
            # while the three re-streamed blocks compute, leaving only a
            # tiny final store on the critical path
            # final stores: the very last piece is a single eb row so the
            # tail store transfer is minimal
            for (e0, e1) in ((0, 4), (4, 8), (8, 12), (12, 15), (15, 16)):
                nc.sync.dma_start(
                    outT.ap().rearrange("(eb p) c -> p eb c", p=128)
                        [:, e0:e1, 7 * TPC:],
                    stage_b2.rearrange("p (eb c) -> p eb c", eb=KB)
                            [:, e0:e1])

    nc.finalize()
    return nc


# ---------------------------------------------------------------- host
def _host_inputs(x, W_q, W_k, W_v, W_o):
    import ml_dtypes
    bf = np.dtype(ml_dtypes.bfloat16)
    xT = np.ascontiguousarray(
        x.reshape(TOK, D).T).astype(bf)                        # [D, TOK]

    # W_o.T tiled (eb, p, dl, e'): row eb*2048 + p*16 + dl, col e'
    woT = np.ascontiguousarray(
        W_o.T.reshape(KB, 128, KB, DK).transpose(2, 1, 0, 3)
        .reshape(KB * D, DK)).astype(bf)

    # RoPE tables, expanded to [DK, S] with interleaved pairs; the sign
    # table carries -sin on even rows, +sin on odd rows.
    i = np.arange(0, DK, 2, dtype=np.float32)
    theta = 1.0 / (ROPE_BASE ** (i / DK))                      # [64]
    pos = np.arange(S, dtype=np.float32)
    freqs = pos[:, None] * theta[None, :]                      # [S, 64]
    cos_t, sin_t = np.cos(freqs), np.sin(freqs)
    ropeC = np.empty((DK, S), np.float32)
    ropeS = np.empty((DK, S), np.float32)
    ropeC[0::2] = cos_t.T
    ropeC[1::2] = cos_t.T
    ropeS[0::2] = -sin_t.T
    ropeS[1::2] = sin_t.T

    # diagonal causal masks: block m (of the 4 key blocks overlapping a
    # 512-query chunk) keeps kk <= qq - 128*m
    kk = np.arange(JB)[:, None]
    qq = np.arange(QCH + 384)[None, :]
    maskd = np.where(kk <= qq - 384, 0.0,
                     MASK_NEG).astype(np.float32).astype(bf)   # [128, 896]

    scale = 1.0 / np.sqrt(np.float32(DK))
    in_maps = []
    for c in range(N_CORES):
        rows = slice(c * HPC * DK, (c + 1) * HPC * DK)
        in_maps.append({
            "xT": xT,
            "wqT": np.ascontiguousarray(
                (W_q[rows] * scale).T).astype(bf),
            "wkT": np.ascontiguousarray(W_k[rows].T).astype(bf),
            "wvT": np.ascontiguousarray(W_v[rows].T).astype(bf),
            "woT": woT,
            "ropeC": ropeC.astype(bf),
            "ropeS": ropeS.astype(bf),
            "maskd": maskd,
        })
    return in_maps


def kernel(x, W_q, W_k, W_v, W_o):
    x = np.asarray(x, dtype=np.float32)
    W_q = np.asarray(W_q, dtype=np.float32)
    W_k = np.asarray(W_k, dtype=np.float32)
    W_v = np.asarray(W_v, dtype=np.float32)
    W_o = np.asarray(W_o, dtype=np.float32)

    if "nc" not in _CACHE:
        _CACHE["nc"] = _build_nc()
    nc = _CACHE["nc"]

    in_maps = _host_inputs(x, W_q, W_k, W_v, W_o)
    res = bass_utils.run_bass_kernel_spmd(
        nc, in_maps, core_ids=list(range(N_CORES)))

    # outT per core: [D, 512] f32; col 64*jc + t -> token 512*jc + 64*c + t
    out_T = np.empty((D, TOK), np.float32)
    for c in range(N_CORES):
        cols = (QCH * np.arange(NCH)[:, None] + TPC * c
                + np.arange(TPC)[None, :]).ravel()
        out_T[:, cols] = res.results[c]["outT"].astype(np.float32)
    return np.ascontiguousarray(out_T.T).reshape(B, S, D).astype(np.float32)

